# revision 1
# baseline (speedup 1.0000x reference)
"""Trainium2 Bass kernel for nn_LocalAttention (5x5 local window attention).

Contract: kernel(**inputs) takes the FULL inputs from setup_inputs() and
returns the FULL output.  Internally shards across 8 NeuronCores as
(batch b in 0..3) x (head-group hg in 0..1, 4 heads each).  Each core
computes a partial output projection; the host sums the two partials per
batch.

Per-core algorithm (validated against the reference in numpy):
  - qT,kT (d-major, fp16) and v (pixel-major, fp16, with ones column for
    the softmax denominator) via fp32r matmuls from host-pre-transposed
    x.T and w slices.
  - k/v live in buffers padded with 2 zero image-rows top+bottom
    (buffer pixel = image pixel + 128): padded neighbors naturally give
    dots=0 -> exp(0)=1 in the denominator and v=0, matching the
    reference's zero-padded local window.
  - Per 256-pixel batch s: banded transposed pairwise dots
    E_T[j, p] = k_buf[s+j] . q[s+p] for j in [0,512) as 4 chunks of 128,
    2-head row-group-packed matmuls (fp16 in, fp32 psum).
  - exp on ACT (scale=1/8) over the valid column slices only; multiply
    by a precomputed window/wrap mask (column-wrapped neighbors are
    masked out and re-added to the denominator via n_pad).
  - Weighted sum over v + denominator via the ones column, one
    accumulating matmul chain per (head, pixel-half).
  - Normalize (DVE reciprocal + per-partition scalar multiply),
    transpose O on the PE, partial out-projection + 0.5*b_out bias.
"""

import numpy as np

B, HMAP, WMAP = 4, 64, 64
N = HMAP * WMAP          # 4096
DIM = 512
HEADS, HEAD_DIM = 8, 64
INNER = HEADS * HEAD_DIM  # 512
SCALE = HEAD_DIM ** -0.5
NB = N + 256             # padded k/v buffer pixels (2 zero rows each side)
NCHUNK = NB // 128       # 34
N_CORES = 8

_cache = {}


def _make_masks():
    """Window/wrap masks for the 4 chunks of a 256-px batch, plus n_pad.

    mask[c, j', p'] = 1 iff o = 128*c + j' - p' - 128 decomposes as
    64*di + dj with |di|,|dj| <= 2 and column p'%64 + dj stays in-image.
    n_pad[p] = number of column-invalid window positions for column p%64.
    """
    o = (128 * np.arange(4)[:, None, None] + np.arange(128)[None, :, None]
         - np.arange(256)[None, None, :] - 128)           # [4,128,256]
    di = np.round(o / 64.0).astype(np.int64)
    dj = o - 64 * di
    col = (np.arange(256) % 64)[None, None, :]
    ok = (np.abs(di) <= 2) & (np.abs(dj) <= 2) & (col + dj >= 0) & (col + dj < 64)
    masks = ok.astype(np.float16)
    colv = np.arange(64)
    npad_col = np.zeros(64, dtype=np.float32)
    for djv in range(-2, 3):
        npad_col += 5.0 * ((colv + djv < 0) | (colv + djv >= 64))
    n_pad = np.tile(npad_col, 2).reshape(128, 1).astype(np.float32)
    return masks, n_pad


def _build_nc(stage=99):
    import os
    stage = int(os.environ.get("KSTAGE", stage))
    import concourse.bass as bass
    import concourse.tile as tile
    from concourse import mybir

    f32 = mybir.dt.float32
    f32r = mybir.dt.float32r
    f16 = mybir.dt.float16
    Exp = mybir.ActivationFunctionType.Exp

    from concourse import bacc
    nc = bacc.Bacc(None, target_bir_lowering=False)
    xt_d = nc.dram_tensor("xt", [DIM, N], f16, kind="ExternalInput")
    wqkvt_d = nc.dram_tensor("wqkvt", [DIM, 768], f16, kind="ExternalInput")
    woutt_d = nc.dram_tensor("woutt", [256, DIM], f16, kind="ExternalInput")
    masks_d = nc.dram_tensor("masks", [4, 128, 256], f16, kind="ExternalInput")
    npad_d = nc.dram_tensor("npad", [128, 1], f32, kind="ExternalInput")
    ident_d = nc.dram_tensor("ident", [128, 128], f32, kind="ExternalInput")
    bhalf_d = nc.dram_tensor("bhalf", [1, DIM], f32, kind="ExternalInput")
    out_d = nc.dram_tensor("out", [N, DIM], f32, kind="ExternalOutput")

    def r32(ap):
        return ap.bitcast(f32r)

    with tile.TileContext(nc) as tc:
        from contextlib import ExitStack
        with ExitStack() as ctx:
            consts = ctx.enter_context(tc.tile_pool(name="consts", bufs=1))

            wqkvt = consts.tile([128, 4, 768], f16)
            nc.sync.dma_start(out=wqkvt,
                              in_=wqkvt_d.rearrange("(c p) m -> p c m", p=128))
            woutt = consts.tile([128, 2, DIM], f16)
            nc.sync.dma_start(out=woutt,
                              in_=woutt_d.rearrange("(c p) m -> p c m", p=128))
            masks = consts.tile([128, 4, 256], f16)
            nc.sync.dma_start(out=masks,
                              in_=masks_d.rearrange("c p f -> p c f"))
            npad = consts.tile([128, 1], f32)
            nc.sync.dma_start(out=npad, in_=npad_d[:, :])
            ident = consts.tile([128, 128], f32)
            nc.sync.dma_start(out=ident, in_=ident_d[:, :])
            bsb = consts.tile([128, DIM], f32)
            nc.gpsimd.dma_start(out=bsb, in_=bhalf_d[:, :].to_broadcast([128, DIM]))

            # persistent activations
            qt = [consts.tile([128, N], f16, tag=f"qt{g}", name=f"qt{g}") for g in range(2)]
            kt = [consts.tile([128, NB], f16, tag=f"kt{g}", name=f"kt{g}") for g in range(2)]
            # v buffer: [p, chunk, 4 heads x (64 + ones col)]
            vsb = consts.tile([128, NCHUNK, 260], f16)

            for g in range(2):
                nc.vector.memset(kt[g][:, 0:128], 0.0)
                nc.vector.memset(kt[g][:, NB - 128:NB], 0.0)
            nc.vector.memset(vsb[:, 0, :], 0.0)
            nc.vector.memset(vsb[:, NCHUNK - 1, :], 0.0)
            # ones columns (after zero memsets of the pad chunks)
            ones_ap = vsb.rearrange("p c (h e) -> p c h e", h=4)[:, :, :, 64:65]
            nc.vector.memset(ones_ap, 1.0)

            # ---------------- Phase B: projections ----------------
            with ExitStack() as bctx:
                psb = bctx.enter_context(
                    tc.tile_pool(name="psum_b", bufs=2, space="PSUM"))
                xin = bctx.enter_context(tc.tile_pool(name="xin", bufs=2))
                xt_view = xt_d.rearrange("(c p) n -> p c n", p=128)
                for blk in range(8):
                    s0 = blk * 512
                    xtile = xin.tile([128, 4, 512], f16)
                    nc.sync.dma_start(out=xtile, in_=xt_view[:, :, s0:s0 + 512])
                    for m in range(4):  # q pair0, q pair1, k pair0, k pair1
                        ps = psb.tile([128, 512], f32, tag="psqk")
                        for kc in range(4):
                            nc.tensor.matmul(
                                ps,
                                wqkvt[:, kc, m * 128:(m + 1) * 128],
                                xtile[:, kc, :],
                                start=(kc == 0), stop=(kc == 3))
                        if m < 2:
                            nc.vector.tensor_copy(qt[m][:, s0:s0 + 512], ps)
                        else:
                            nc.vector.tensor_copy(
                                kt[m - 2][:, 128 + s0:128 + s0 + 512], ps)
                    for sub in range(4):
                        psv = psb.tile([128, 256], f32, tag="psv")
                        for kc in range(4):
                            nc.tensor.matmul(
                                psv,
                                xtile[:, kc, sub * 128:(sub + 1) * 128],
                                wqkvt[:, kc, 512:768],
                                start=(kc == 0), stop=(kc == 3))
                        ci = 1 + blk * 4 + sub
                        nc.vector.tensor_copy(
                            vsb[:, ci].rearrange("p (h e) -> p h e", h=4)[:, :, 0:64],
                            psv.rearrange("p (h e) -> p h e", h=4))

            if stage < 2:
                # dump q instead of attention output
                dbg = consts.tile([128, DIM], f32)
                nc.vector.tensor_copy(dbg, qt[0][:, 0:DIM])
                for t in range(32):
                    nc.sync.dma_start(out=out_d[t * 128:(t + 1) * 128, :], in_=dbg)

            tc.strict_bb_all_engine_barrier()

            # ---------------- Phase C/D: attention + projection ----------------
            with ExitStack() as cctx:
              if stage >= 2:
                  pspw = cctx.enter_context(
                      tc.tile_pool(name="psum_pw", bufs=2, space="PSUM"))
                  pso = cctx.enter_context(
                      tc.tile_pool(name="psum_o", bufs=1, space="PSUM"))
                  pst = cctx.enter_context(
                      tc.tile_pool(name="psum_t", bufs=1, space="PSUM"))
                  pspj = cctx.enter_context(
                      tc.tile_pool(name="psum_pj", bufs=1, space="PSUM"))
                  epool = cctx.enter_context(tc.tile_pool(name="em", bufs=2))
                  erpool = cctx.enter_context(tc.tile_pool(name="er", bufs=2))
                  dpool = cctx.enter_context(tc.tile_pool(name="den", bufs=2))
                  opool = cctx.enter_context(tc.tile_pool(name="opix", bufs=2))
                  otpool = cctx.enter_context(tc.tile_pool(name="ot", bufs=2))
                  obpool = cctx.enter_context(tc.tile_pool(name="ob", bufs=2))

                  for si in range(16):
                      s = si * 256
                      # slot order (hs, g): concurrent row-group matmuls (hs=0
                      # vs hs=1) must land in different PSUM banks.
                      em = epool.tile([128, 4, 4, 256], f16)  # [j, chunk, slot, p']
                      for c in range(4):
                          pw = pspw.tile([128, 2, 2, 256], f32)
                          for g in range(2):
                              for hs in range(2):
                                  lo_p, hi_p = hs * 64, (hs + 1) * 64
                                  nc.tensor.matmul(
                                      pw[:, hs, g, :],
                                      kt[g][lo_p:hi_p, s + 128 * c:s + 128 * c + 128],
                                      qt[g][lo_p:hi_p, s:s + 256],
                                      start=True, stop=True)
                          er = erpool.tile([128, 4, 256], f16)
                          if c == 0:
                              lo, hi = 0, 130
                              nc.vector.memset(er[:, :, 130:256], 0.0)
                          elif c == 3:
                              lo, hi = 126, 256
                              nc.vector.memset(er[:, :, 0:126], 0.0)
                          else:
                              lo, hi = 0, 256
                          nc.scalar.activation(
                              out=er[:, :, lo:hi],
                              in_=pw.rearrange("p a b f -> p (a b) f")[:, :, lo:hi],
                              func=Exp, scale=SCALE)
                          mask_b = masks[:, c, :].unsqueeze(1).to_broadcast(
                              [128, 4, 256])
                          nc.gpsimd.tensor_mul(em[:, c], er, mask_b)

                      if stage < 3:
                          continue
                      po = [pso.tile([128, 4, 128], f32, tag=f"po{ph}",
                                     name=f"po{ph}") for ph in range(2)]
                      for gh in range(4):
                          for ph in range(2):
                              for c in range(4):
                                  slot = 2 * (gh % 2) + gh // 2
                                  nc.tensor.matmul(
                                      po[ph][:, gh, 0:65],
                                      em[:, c, slot, ph * 128:(ph + 1) * 128],
                                      vsb[:, 2 * si + c, 65 * gh:65 * gh + 65],
                                      start=(c == 0), stop=(c == 3))
                      den = dpool.tile([128, 2, 4], f32, tag="den")
                      for ph in range(2):
                          nc.vector.tensor_add(
                              den[:, ph, :].unsqueeze(2),
                              po[ph][:, :, 64:65],
                              npad.unsqueeze(2).to_broadcast([128, 4, 1]))
                      rec = dpool.tile([128, 2, 4], f32, tag="rec")
                      nc.vector.reciprocal(rec, den)

                      for ph in range(2):
                          opix = opool.tile([128, 256], f32)
                          for gh in range(4):
                              nc.vector.tensor_scalar_mul(
                                  opix[:, gh * 64:(gh + 1) * 64],
                                  po[ph][:, gh, 0:64],
                                  rec[:, ph, gh:gh + 1])
                          if stage < 4:
                              continue
                          otb = otpool.tile([128, 2, 128], f16)
                          for i in range(2):
                              pt = pst.tile([128, 128], f32)
                              nc.tensor.transpose(
                                  pt, opix[:, i * 128:(i + 1) * 128], ident)
                              nc.vector.tensor_copy(otb[:, i], pt)
                          pj = pspj.tile([128, DIM], f32)
                          for i in range(2):
                              nc.tensor.matmul(
                                  pj, otb[:, i], woutt[:, i],
                                  start=(i == 0), stop=(i == 1))
                          ob = obpool.tile([128, DIM], f32)
                          nc.vector.tensor_add(ob, pj, bsb)
                          px = s + ph * 128
                          nc.sync.dma_start(out=out_d[px:px + 128, :], in_=ob)

    nc.finalize()
    return nc


def _prepare_core_inputs(x, w_qkv, w_out, b_out):
    masks, n_pad = _make_masks()
    ident = np.eye(128, dtype=np.float32)
    bhalf = (0.5 * b_out).reshape(1, DIM).astype(np.float32)
    per_core = []
    for ci in range(N_CORES):
        b, hg = ci // 2, ci % 2
        q_rows = w_qkv[256 * hg:256 * hg + 256]
        k_rows = w_qkv[INNER + 256 * hg:INNER + 256 * hg + 256]
        v_rows = w_qkv[2 * INNER + 256 * hg:2 * INNER + 256 * hg + 256]
        w_slice = np.concatenate([q_rows, k_rows, v_rows], axis=0)  # [768, 512]
        per_core.append({
            "xt": np.ascontiguousarray(x[b].T).astype(np.float16),
            "wqkvt": np.ascontiguousarray(w_slice.T).astype(np.float16),
            "woutt": np.ascontiguousarray(
                w_out[:, 256 * hg:256 * hg + 256].T).astype(np.float16),
            "masks": masks,
            "npad": n_pad,
            "ident": ident,
            "bhalf": bhalf,
        })
    return per_core


def kernel(x, w_qkv, w_out, b_out, h, w):
    assert int(h) == HMAP and int(w) == WMAP
    x = np.asarray(x, dtype=np.float32)
    w_qkv = np.asarray(w_qkv, dtype=np.float32)
    w_out = np.asarray(w_out, dtype=np.float32)
    b_out = np.asarray(b_out, dtype=np.float32)

    if "nc" not in _cache:
        _cache["nc"] = _build_nc()
    nc = _cache["nc"]

    from concourse.bass_utils import run_bass_kernel_spmd
    in_maps = _prepare_core_inputs(x, w_qkv, w_out, b_out)
    res = run_bass_kernel_spmd(nc, in_maps, core_ids=list(range(N_CORES)))
    out = np.zeros((B, N, DIM), dtype=np.float32)
    for b in range(B):
        out[b] = res.results[2 * b]["out"] + res.results[2 * b + 1]["out"]
    return out



# revision 14
# speedup vs baseline: 1.4333x; 1.4333x over previous
"""Trainium2 Bass kernel for nn_LocalAttention (5x5 local window attention).

Contract: kernel(**inputs) takes the FULL inputs from setup_inputs() and
returns the FULL output.  Internally shards across 8 NeuronCores as
(batch b in 0..3) x (head-group hg in 0..1, 4 heads each).  Each core
computes a partial output projection; the host sums the two partials per
batch.

Per-core algorithm (validated against the reference in numpy):
  - qT,kT (d-major, fp16) and v (pixel-major, fp16, with ones column for
    the softmax denominator) via fp32r matmuls from host-pre-transposed
    x.T and w slices.
  - k/v live in buffers padded with 2 zero image-rows top+bottom
    (buffer pixel = image pixel + 128): padded neighbors naturally give
    dots=0 -> exp(0)=1 in the denominator and v=0, matching the
    reference's zero-padded local window.
  - Per 256-pixel batch s: banded transposed pairwise dots
    E_T[j, p] = k_buf[s+j] . q[s+p] for j in [0,512) as 4 chunks of 128,
    2-head row-group-packed matmuls (fp16 in, fp32 psum).  Edge chunks
    (c=0, c=3) only touch one 128-px half of the batch (the rest of the
    band is column-masked), so they run at N=128.
  - exp on ACT (scale=1/8) over the live column slices only; DVE
    multiplies by a precomputed window/wrap mask (column-wrapped
    neighbors are masked out and re-added to the denominator via n_pad).
  - Weighted sum over v + denominator via the ones column, one
    3-chunk accumulating matmul chain per (head, pixel-half).
  - Normalize (DVE reciprocal + one broadcast tensor-tensor mul per
    half), transpose O on the PE, partial out-projection + 0.5*b_out
    bias, fp16 DMA out (host sums the two partials per batch in fp32).
"""

import numpy as np

B, HMAP, WMAP = 4, 64, 64
N = HMAP * WMAP          # 4096
DIM = 512
HEADS, HEAD_DIM = 8, 64
INNER = HEADS * HEAD_DIM  # 512
SCALE = HEAD_DIM ** -0.5
NB = N + 256             # padded k/v buffer pixels (2 zero rows each side)
NCHUNK = NB // 128       # 34
N_CORES = 8

_cache = {}


def _make_masks():
    """Window/wrap masks for the 4 chunks of a 256-px batch, plus n_pad.

    mask[c, j', p'] = 1 iff o = 128*c + j' - p' - 128 decomposes as
    64*di + dj with |di|,|dj| <= 2 and column p'%64 + dj stays in-image.
    n_pad[p] = number of column-invalid window positions for column p%64.
    """
    o = (128 * np.arange(4)[:, None, None] + np.arange(128)[None, :, None]
         - np.arange(256)[None, None, :] - 128)           # [4,128,256]
    di = np.round(o / 64.0).astype(np.int64)
    dj = o - 64 * di
    col = (np.arange(256) % 64)[None, None, :]
    ok = (np.abs(di) <= 2) & (np.abs(dj) <= 2) & (col + dj >= 0) & (col + dj < 64)
    masks = ok.astype(np.float16)
    colv = np.arange(64)
    npad_col = np.zeros(64, dtype=np.float32)
    for djv in range(-2, 3):
        npad_col += 5.0 * ((colv + djv < 0) | (colv + djv >= 64))
    n_pad = np.tile(npad_col, 2).reshape(128, 1).astype(np.float32)
    return masks, n_pad


def _build_nc(stage=99):
    import os
    stage = int(os.environ.get("KSTAGE", stage))
    import concourse.bass as bass
    import concourse.tile as tile
    from concourse import mybir

    f32 = mybir.dt.float32
    f32r = mybir.dt.float32r
    f16 = mybir.dt.float16
    Exp = mybir.ActivationFunctionType.Exp

    from concourse import bacc
    nc = bacc.Bacc(None, target_bir_lowering=False)
    # xt/wqkvt/masks come pre-blocked from the host so every DMA descriptor
    # is a contiguous >=2KB per-partition run (1KB descriptors measured at
    # ~60% of DMA line rate during the startup-critical window).
    xt_d = nc.dram_tensor("xt", [8, 128, 4, 512], f16, kind="ExternalInput")
    wqkvt_d = nc.dram_tensor("wqkvt", [128, 4, 768], f16, kind="ExternalInput")
    woutt_d = nc.dram_tensor("woutt", [256, DIM], f16, kind="ExternalInput")
    masks_d = nc.dram_tensor("masks", [128, 4, 256], f16, kind="ExternalInput")
    npad_d = nc.dram_tensor("npad", [128, 1], f32, kind="ExternalInput")
    ident_d = nc.dram_tensor("ident", [128, 128], f16, kind="ExternalInput")
    bhalf_d = nc.dram_tensor("bhalf", [1, DIM], f32, kind="ExternalInput")
    out_d = nc.dram_tensor("out", [N, DIM], f16, kind="ExternalOutput")

    with tile.TileContext(nc) as tc:
        from contextlib import ExitStack
        with ExitStack() as ctx:
            consts = ctx.enter_context(tc.tile_pool(name="consts", bufs=1))

            # Only the qkv weights block the first matmul; everything else
            # (phase-C constants) is DMA'd after the phase-B loop has been
            # issued so the first xtile load isn't stuck behind them in the
            # HWDGE queue.
            wqkvt = consts.tile([128, 4, 768], f16)
            nc.sync.dma_start(out=wqkvt, in_=wqkvt_d[:, :, :])
            woutt = consts.tile([128, 2, DIM], f16)
            masks = consts.tile([128, 4, 256], f16)
            # c=1,2 masks replicated across the 4 head slots (dense operand
            # for the gpsimd mask multiply — broadcast APs run ~2x slower
            # there).
            masksf = consts.tile([128, 2, 4, 256], f16)
            npad = consts.tile([128, 1], f32)
            ident = consts.tile([128, 128], f16)
            bsb = consts.tile([128, DIM], f32)

            # persistent activations
            qt = [consts.tile([128, N], f16, tag=f"qt{g}", name=f"qt{g}") for g in range(2)]
            kt = [consts.tile([128, NB], f16, tag=f"kt{g}", name=f"kt{g}") for g in range(2)]
            # v buffer: [p, chunk, 4 heads x (64 + ones col)]
            vsb = consts.tile([128, NCHUNK, 260], f16)

            for g in range(2):
                nc.vector.memset(kt[g][:, 0:128], 0.0)
                nc.vector.memset(kt[g][:, NB - 128:NB], 0.0)
            nc.vector.memset(vsb[:, 0, :], 0.0)
            nc.vector.memset(vsb[:, NCHUNK - 1, :], 0.0)
            # ones columns (after zero memsets of the pad chunks)
            ones_ap = vsb.rearrange("p c (h e) -> p c h e", h=4)[:, :, :, 64:65]
            nc.vector.memset(ones_ap, 1.0)

            # ---------------- Phase B: projections ----------------
            with ExitStack() as bctx:
                psb = bctx.enter_context(
                    tc.tile_pool(name="psum_b", bufs=2, space="PSUM"))
                xin = bctx.enter_context(tc.tile_pool(name="xin", bufs=2))
                for blk in range(8):
                    s0 = blk * 512
                    xtile = xin.tile([128, 4, 512], f16)
                    nc.sync.dma_start(out=xtile, in_=xt_d[blk])
                    for m in range(4):  # q pair0, q pair1, k pair0, k pair1
                        ps = psb.tile([128, 512], f32, tag="psqk")
                        for kc in range(4):
                            nc.tensor.matmul(
                                ps,
                                wqkvt[:, kc, m * 128:(m + 1) * 128],
                                xtile[:, kc, :],
                                start=(kc == 0), stop=(kc == 3))
                        if m < 2:
                            nc.scalar.copy(qt[m][:, s0:s0 + 512], ps)
                        else:
                            nc.scalar.copy(
                                kt[m - 2][:, 128 + s0:128 + s0 + 512], ps)
                    for sub in range(4):
                        psv = psb.tile([128, 256], f32, tag="psv")
                        for kc in range(4):
                            nc.tensor.matmul(
                                psv,
                                xtile[:, kc, sub * 128:(sub + 1) * 128],
                                wqkvt[:, kc, 512:768],
                                start=(kc == 0), stop=(kc == 3))
                        ci = 1 + blk * 4 + sub
                        nc.vector.tensor_copy(
                            vsb[:, ci].rearrange("p (h e) -> p h e", h=4)[:, :, 0:64],
                            psv.rearrange("p (h e) -> p h e", h=4))

            # phase-C constants: separate HWDGE queue (nc.scalar) so they
            # never sit ahead of the startup-critical xtile loads.
            nc.scalar.dma_start(out=woutt,
                                in_=woutt_d.rearrange("(c p) m -> p c m", p=128))
            nc.scalar.dma_start(out=masks, in_=masks_d[:, :, :])
            nc.scalar.dma_start(out=npad, in_=npad_d[:, :])
            nc.scalar.dma_start(out=ident, in_=ident_d[:, :])
            nc.gpsimd.dma_start(out=bsb, in_=bhalf_d[:, :].to_broadcast([128, DIM]))
            for ci2 in range(2):
                nc.gpsimd.tensor_copy(
                    masksf[:, ci2],
                    masks[:, 1 + ci2].unsqueeze(1).to_broadcast([128, 4, 256]))

            if stage < 2:
                # dump q instead of attention output
                dbg = consts.tile([128, DIM], f16)
                nc.vector.tensor_copy(dbg, qt[0][:, 0:DIM])
                for t in range(32):
                    nc.sync.dma_start(out=out_d[t * 128:(t + 1) * 128, :], in_=dbg)

            # (no barrier: Tile's dependency tracking lets the first si
            # iterations overlap the tail of phase B)

            # ---------------- Phase C/D: attention + projection ----------------
            with ExitStack() as cctx:
              if stage >= 2:
                  pspw = cctx.enter_context(
                      tc.tile_pool(name="psum_pw", bufs=2, space="PSUM"))
                  pso = cctx.enter_context(
                      tc.tile_pool(name="psum_o", bufs=1, space="PSUM"))
                  pst = cctx.enter_context(
                      tc.tile_pool(name="psum_t", bufs=1, space="PSUM"))
                  pspj = cctx.enter_context(
                      tc.tile_pool(name="psum_pj", bufs=1, space="PSUM"))
                  epool = cctx.enter_context(tc.tile_pool(name="em", bufs=2))
                  erpool = cctx.enter_context(tc.tile_pool(name="er", bufs=2))
                  dpool = cctx.enter_context(tc.tile_pool(name="den", bufs=2))
                  opool = cctx.enter_context(tc.tile_pool(name="opix", bufs=2))
                  otpool = cctx.enter_context(tc.tile_pool(name="ot", bufs=2))
                  obpool = cctx.enter_context(tc.tile_pool(name="ob", bufs=2))

                  for si in range(16):
                      s = si * 256
                      # em: [j, chunk, slot, p'].  Edge chunks only live on
                      # one pixel-half; the dead half is never written nor
                      # read (po skips c=3 for ph=0 and c=0 for ph=1).
                      em = epool.tile([128, 4, 4, 256], f16)
                      for c in range(4):
                          lo, hi = (0, 128) if c == 0 else \
                                   (128, 256) if c == 3 else (0, 256)
                          # slot order (hs, g): concurrent row-group matmuls
                          # (hs=0 vs hs=1) must land in different PSUM banks.
                          pw = pspw.tile([128, 2, 2, 256], f32)
                          for g in range(2):
                              for hs in range(2):
                                  lo_p, hi_p = hs * 64, (hs + 1) * 64
                                  nc.tensor.matmul(
                                      pw[:, hs, g, lo:hi],
                                      kt[g][lo_p:hi_p, s + 128 * c:s + 128 * c + 128],
                                      qt[g][lo_p:hi_p, s + lo:s + hi],
                                      start=True, stop=True)
                          er = erpool.tile([128, 4, 256], f16)
                          nc.scalar.activation(
                              out=er[:, :, lo:hi],
                              in_=pw.rearrange("p a b f -> p (a b) f")[:, :, lo:hi],
                              func=Exp, scale=SCALE)
                          if c in (1, 2):
                              nc.vector.tensor_mul(
                                  em[:, c], er, masksf[:, c - 1])
                          else:
                              mask_b = masks[:, c, lo:hi].unsqueeze(1) \
                                  .to_broadcast([128, 4, hi - lo])
                              nc.vector.tensor_mul(
                                  em[:, c, :, lo:hi], er[:, :, lo:hi], mask_b)

                      if stage < 3:
                          continue
                      po = [pso.tile([128, 4, 65], f32, tag=f"po{ph}",
                                     name=f"po{ph}") for ph in range(2)]
                      for gh in range(4):
                          slot = 2 * (gh % 2) + gh // 2
                          for ph in range(2):
                              cs = (0, 1, 2) if ph == 0 else (1, 2, 3)
                              for i, c in enumerate(cs):
                                  nc.tensor.matmul(
                                      po[ph][:, gh, 0:65],
                                      em[:, c, slot, ph * 128:(ph + 1) * 128],
                                      vsb[:, 2 * si + c, 65 * gh:65 * gh + 65],
                                      start=(i == 0), stop=(i == 2))
                      den = dpool.tile([128, 2, 4], f32, tag="den")
                      for ph in range(2):
                          nc.vector.tensor_add(
                              den[:, ph, :].unsqueeze(2),
                              po[ph][:, :, 64:65],
                              npad.unsqueeze(2).to_broadcast([128, 4, 1]))
                      rec = dpool.tile([128, 2, 4], f32, tag="rec")
                      nc.vector.reciprocal(rec, den)

                      for ph in range(2):
                          opix = opool.tile([128, 256], f16)
                          nc.vector.tensor_mul(
                              opix.rearrange("p (g e) -> p g e", g=4),
                              po[ph][:, :, 0:64],
                              rec[:, ph, :].unsqueeze(2).to_broadcast(
                                  [128, 4, 64]))
                          if stage < 4:
                              continue
                          otb = otpool.tile([128, 2, 128], f16)
                          for i in range(2):
                              pt = pst.tile([128, 128], f16)
                              nc.tensor.transpose(
                                  pt, opix[:, i * 128:(i + 1) * 128], ident)
                              if i == 0:
                                  nc.vector.tensor_copy(otb[:, i], pt)
                              else:
                                  nc.scalar.copy(otb[:, i], pt)
                          pj = pspj.tile([128, DIM], f32)
                          for i in range(2):
                              nc.tensor.matmul(
                                  pj, otb[:, i], woutt[:, i],
                                  start=(i == 0), stop=(i == 1))
                          ob = obpool.tile([128, DIM], f16)
                          nc.vector.tensor_add(ob, pj, bsb)
                          px = s + ph * 128
                          nc.sync.dma_start(out=out_d[px:px + 128, :], in_=ob)

    nc.finalize()
    return nc


def _prepare_core_inputs(x, w_qkv, w_out, b_out):
    masks, n_pad = _make_masks()
    masks_p = np.ascontiguousarray(masks.transpose(1, 0, 2))  # [128, 4, 256]
    ident = np.eye(128, dtype=np.float16)
    bhalf = (0.5 * b_out).reshape(1, DIM).astype(np.float32)
    per_core = []
    for ci in range(N_CORES):
        b, hg = ci // 2, ci % 2
        q_rows = w_qkv[256 * hg:256 * hg + 256]
        k_rows = w_qkv[INNER + 256 * hg:INNER + 256 * hg + 256]
        v_rows = w_qkv[2 * INNER + 256 * hg:2 * INNER + 256 * hg + 256]
        w_slice = np.concatenate([q_rows, k_rows, v_rows], axis=0)  # [768, 512]
        xt = x[b].T.astype(np.float16)                       # [512, 4096]
        # blocked: [blk, p, c, n-slice] so each (partition) DMA run is 4KB
        xtb = np.ascontiguousarray(
            xt.reshape(4, 128, 8, 512).transpose(2, 1, 0, 3))
        wq = np.ascontiguousarray(
            w_slice.T.astype(np.float16).reshape(4, 128, 768).transpose(1, 0, 2))
        per_core.append({
            "xt": xtb,
            "wqkvt": wq,
            "woutt": np.ascontiguousarray(
                w_out[:, 256 * hg:256 * hg + 256].T).astype(np.float16),
            "masks": masks_p,
            "npad": n_pad,
            "ident": ident,
            "bhalf": bhalf,
        })
    return per_core


def kernel(x, w_qkv, w_out, b_out, h, w):
    assert int(h) == HMAP and int(w) == WMAP
    x = np.asarray(x, dtype=np.float32)
    w_qkv = np.asarray(w_qkv, dtype=np.float32)
    w_out = np.asarray(w_out, dtype=np.float32)
    b_out = np.asarray(b_out, dtype=np.float32)

    if "nc" not in _cache:
        _cache["nc"] = _build_nc()
    nc = _cache["nc"]

    from concourse.bass_utils import run_bass_kernel_spmd
    in_maps = _prepare_core_inputs(x, w_qkv, w_out, b_out)
    res = run_bass_kernel_spmd(nc, in_maps, core_ids=list(range(N_CORES)))
    out = np.zeros((B, N, DIM), dtype=np.float32)
    for b in range(B):
        out[b] = (res.results[2 * b]["out"].astype(np.float32)
                  + res.results[2 * b + 1]["out"].astype(np.float32))
    return out
